# revision 20
# baseline (speedup 1.0000x reference)
"""Causal self-attention for (2, 2048, 1024), 16 heads, on 8 trn2 cores.

Sharding: batch x head-group. Core c handles batch b = c // 4 and heads
[4*(c%4), 4*(c%4)+4). Each core computes q/k/v projections for its 4 heads
from the (host-pre-transposed) hidden states of its batch, runs causal
attention per head fully in transposed layout, applies its slice of the
output projection, and returns a [2048, 1024] bf16 partial. The host sums
the 4 partials per batch and adds the exact bias term bv @ Wo + bo.

Bias algebra: bk is softmax-shift-invariant (dropped exactly); bv commutes
through the probability-weighted average and the output projection, so it
folds, with bo, into a constant row added on the host. Only bq must be
applied on-device (to Q, via a fused DVE bias-add on the PSUM->SBUF copy).

Engine placement: PE does projections/scores/ctx/rowsum-broadcast/out-proj;
Scalar does exp (and K copies); DVE does Q bias copies + fast reciprocal;
GpSimd (Pool) does V copies, causal-mask multiplies, normalize multiplies,
and half the output copies. The softmax rowsum rides along as a ones column
in the ctx matmul stationary operand; its reciprocal is broadcast into the
unused partitions 64..127 of the same PSUM bank by a tiny PE matmul.
"""

import sys

sys.path.insert(0, "/opt/trn_rl_repo")

import ml_dtypes
import numpy as np

import concourse.bass as bass
from concourse.bass import _add_dep_helper
import concourse.mybir as mybir
import concourse.tile as tile
from concourse.vector_clock import ScopedClock

B, S, H, NH, HD = 2, 2048, 1024, 16, 64
NCORES = 8
HPC = 4          # heads per core
CHUNK = 512      # i-chunk width (PSUM bank)
NIT = S // 128   # 16 j/i-tiles (128 each)
NIC = S // CHUNK # 4 i-chunks
KT = H // 128    # 8 contraction tiles for projections
SCALE = 1.0 / np.sqrt(HD)

f32 = mybir.dt.float32
f32r = mybir.dt.float32r
bf16 = mybir.dt.bfloat16
EXP = mybir.ActivationFunctionType.Exp
COPY = mybir.ActivationFunctionType.Copy
MUL = mybir.AluOpType.mult
ADD = mybir.AluOpType.add


class _TC(tile.TileContext):
    """TileContext whose tail drain carries no sem waits: this walrus build
    rejects instructions with more than one sync-wait command, so the waits
    are emitted as individual wait_ge instructions instead."""

    def _drain_and_barrier(self, tick_clock, wait_clock):
        nc = self.nc
        carrier = nc.sync.nop()
        wait_clock.add_sem_waits(
            carrier.ins, ScopedClock({None: tick_clock.global_clock})
        )
        si = carrier.ins.sync_info
        waits = list(si.on_wait) if si and si.on_wait else []
        si.on_wait = []
        assert self.sems is not None
        id2handle = {h.num: h for h in self.sems.allocated().values()}
        for w in waits:
            nc.sync.wait_ge(id2handle[w.id], w.wait_value)
        nc.sync.drain()
        nc.all_engine_barrier()
        popped = nc._tile_sem_poison_stack.pop()
        assert popped is self._sem_poison
        nc.clear_and_free_semaphores(list(self.sems.allocated().values()))
        nc.all_engine_barrier()


_waitfix_ctr = [0]


def _split_multiwaits(nc):
    """Hoist all-but-one sync wait off every instruction into standalone
    single-wait EventSemaphore instructions (same engine, same position)."""
    for f in nc.m.functions:
        for bb in f.blocks:
            out = []
            changed = False
            for inst in bb.instructions:
                si = inst.sync_info
                waits = list(si.on_wait) if si and si.on_wait else []
                if len(waits) > 1:
                    changed = True
                    for w in waits[:-1]:
                        _waitfix_ctr[0] += 1
                        ev = mybir.InstEventSemaphore(
                            name=f"I-waitfix-{_waitfix_ctr[0]}",
                            engine=inst.engine,
                            ins=[],
                            outs=[],
                            sync_info=mybir.SyncInfo(on_wait=[w], on_update=[]),
                        )
                        nc.register_instruction(ev)
                        out.append(ev)
                    si.on_wait = waits[-1:]
                out.append(inst)
            if changed:
                bb.instructions = out


def _build_program():
    nc = bass.Bass("TRN2", target_bir_lowering=False, debug=False,
                   num_devices=NCORES)

    xt = nc.dram_tensor("xt", [H, S], bf16, kind="ExternalInput")
    wq = nc.dram_tensor("wq", [H, HPC * HD], bf16, kind="ExternalInput")
    wk = nc.dram_tensor("wk", [H, HPC * HD], bf16, kind="ExternalInput")
    wv = nc.dram_tensor("wv", [H, HPC * HD], bf16, kind="ExternalInput")
    wo = nc.dram_tensor("wo", [HPC * HD, H], bf16, kind="ExternalInput")
    bq2 = nc.dram_tensor("bq2", [128, 2], f32, kind="ExternalInput")
    ones64 = nc.dram_tensor("ones64", [1, 64], f32, kind="ExternalInput")
    mask = nc.dram_tensor("mask", [128, 128], bf16, kind="ExternalInput")
    onescol = nc.dram_tensor("onescol", [128, NIT * HPC], bf16,
                             kind="ExternalInput")
    outp = nc.dram_tensor("outp", [S, H], bf16, kind="ExternalOutput")

    last_pe = [None]

    def _mm(inst):
        if last_pe[0] is not None:
            _add_dep_helper(inst.ins, last_pe[0].ins, sync=False,
                            reason="pe emission order")
        last_pe[0] = inst
        return inst

    with _TC(nc) as tc:
        with (
            tc.tile_pool(name="const", bufs=1) as constp,
            tc.tile_pool(name="xtp", bufs=1) as xtp,
            tc.tile_pool(name="wp", bufs=1) as wp,
            tc.tile_pool(name="qk", bufs=1) as qkp,
            tc.tile_pool(name="vj", bufs=1) as vjp,
            tc.tile_pool(name="ctxT2", bufs=1) as ctxT2p,
        ):
            # ---- constants + inputs ---------------------------------------
            ones64_sb = constp.tile([1, 64], f32r)
            nc.sync.dma_start(ones64_sb[:], ones64.ap().bitcast(f32r))
            mask_sb = constp.tile([128, 128], bf16)
            nc.sync.dma_start(mask_sb[:], mask.ap())
            bq2_sb = constp.tile([128, 2], f32)
            nc.sync.dma_start(bq2_sb[:], bq2.ap())

            wv_sb = wp.tile([128, KT, HPC * HD], bf16, tag="wv")
            nc.sync.dma_start(
                wv_sb[:], wv.ap().rearrange("(t p) m -> p t m", p=128))
            xt_sb = xtp.tile([128, KT, S], bf16)
            for t in range(KT):
                nc.sync.dma_start(
                    xt_sb[:, t, :], xt.ap()[t * 128:(t + 1) * 128, :])
            wk_sb = wp.tile([128, KT, HPC * HD], bf16, tag="wk")
            nc.sync.dma_start(
                wk_sb[:], wk.ap().rearrange("(t p) m -> p t m", p=128))
            wq_sb = wp.tile([128, KT, HPC * HD], bf16, tag="wq")
            nc.sync.dma_start(
                wq_sb[:], wq.ap().rearrange("(t p) m -> p t m", p=128))
            wo_sb = wp.tile([128, 2, H], bf16, tag="wo")
            nc.sync.dma_start(
                wo_sb[:], wo.ap().rearrange("(p k) n -> k p n", k=128))

            qt_sb = qkp.tile([128, 2, S], bf16, tag="qt")
            kt_sb = qkp.tile([128, 2, S], bf16, tag="kt")
            # v'[j, d] packed [j-in-tile, j-tile, head, d|1]; col 64 = ones
            vj_sb = vjp.tile([128, NIT, HPC, HD + 1], bf16)
            nc.sync.dma_start(vj_sb[:, :, :, HD:HD + 1], onescol.ap())

            # normalized ctx^T, packed [d-in-pair, pair, i]
            ctxT2_sb = ctxT2p.tile([128, 2, S], bf16)

            # ---- phase A: K/Q for pair 0, then V --------------------------
            # K/Q d-tile 0 kt-outer over 8 banks (compute starts after the
            # first xt tile lands); V jt-outer (one bank per j-tile, copies
            # on the still-idle scalar engine). K/Q d-tile 1 is emitted
            # later, as PE filler inside the attention stream.
            with tc.tile_pool(name="p1", bufs=8, space="PSUM") as p1p:
                pss = [p1p.tile([128, CHUNK], f32, tag="p1",
                                name=f"mm_kq0_{i}")
                       for i in range(8)]
                units0 = [("k", wk_sb, kt_sb, s) for s in range(NIC)]
                units0 += [("q", wq_sb, qt_sb, s) for s in range(NIC)]
                for t in range(KT):
                    for i, (name, w_sb, dst, sc) in enumerate(units0):
                        _mm(nc.tensor.matmul(
                            pss[i][:],
                            w_sb[:, t, 0:128],
                            xt_sb[:, t, sc * CHUNK:(sc + 1) * CHUNK],
                            start=(t == 0),
                            stop=(t == KT - 1),
                        ))
                for i, (name, w_sb, dst, sc) in enumerate(units0):
                    o = dst[:, 0, sc * CHUNK:(sc + 1) * CHUNK]
                    if name == "k":
                        nc.scalar.activation(o, pss[i][:], COPY)
                    else:
                        nc.vector.tensor_scalar(
                            out=o, in0=pss[i][:],
                            scalar1=bq2_sb[:, 0:1],
                            scalar2=None, op0=ADD)
                for jt in range(NIT):
                    vps = p1p.tile([128, CHUNK], f32, tag="p1",
                                   name=f"vmm_{jt}")
                    for t in range(KT):
                        _mm(nc.tensor.matmul(
                            vps[:, 0:HPC * HD],
                            xt_sb[:, t, jt * 128:(jt + 1) * 128],
                            wv_sb[:, t, :],
                            start=(t == 0),
                            stop=(t == KT - 1),
                        ))
                    nc.scalar.activation(
                        vj_sb[:, jt, :, 0:HD],
                        vps[:, 0:HPC * HD].rearrange(
                            "p (h d) -> p h d", h=HPC),
                        COPY,
                    )

            # ---- phase B: attention (+ K/Q d-tile-1 as PE filler) ---------
            # 16 global "groups" (4 heads x 4 j-tile groups). Per group:
            # scores+exp for 4 j-tiles, one K/Q-d1 filler unit (first 8
            # groups), the one-group-deferred normalize of the previous
            # chunk, and the ctx accumulation of the PREVIOUS group's chunk
            # (lag 1, chunk-serial over a 2-bank ring reading SBUF pt
            # tiles). The lag keeps the PE dense so it stays at full clock,
            # with the exp stream as the pacing resource.
            pending_norms = []

            def flush_norm(upto_key):
                while pending_norms and pending_norms[0][0] <= upto_key:
                    _, fin = pending_norms.pop(0)
                    fin()

            filler_units = []

            def make_kq1(name, w_sb, dst, sc):
                def emit():
                    ps = fillp.tile([128, CHUNK], f32, tag="fill",
                                    name=f"f_{name}{sc}")
                    for t in range(KT):
                        _mm(nc.tensor.matmul(
                            ps[:],
                            w_sb[:, t, 128:256],
                            xt_sb[:, t, sc * CHUNK:(sc + 1) * CHUNK],
                            start=(t == 0),
                            stop=(t == KT - 1),
                        ))
                    o = dst[:, 1, sc * CHUNK:(sc + 1) * CHUNK]
                    if name == "k":
                        nc.vector.tensor_copy(o, ps[:])
                    else:
                        nc.vector.tensor_scalar(
                            out=o, in0=ps[:],
                            scalar1=bq2_sb[:, 1:2],
                            scalar2=None, op0=ADD)
                return emit

            for name, w_sb, dst in (("k", wk_sb, kt_sb), ("q", wq_sb, qt_sb)):
                for sc_ in range(NIC):
                    filler_units.append(make_kq1(name, w_sb, dst, sc_))

            ptp_cm = tc.tile_pool(name="pt", bufs=26)
            ptp = ptp_cm.__enter__()
            rsp_cm = tc.tile_pool(name="rs", bufs=3)
            rsp = rsp_cm.__enter__()
            bcsbp_cm = tc.tile_pool(name="bcsb", bufs=2)
            bcsbp = bcsbp_cm.__enter__()
            ctxp_cm = tc.tile_pool(name="ctx", bufs=2, space="PSUM")
            ctxp = ctxp_cm.__enter__()
            bcp_cm = tc.tile_pool(name="bc", bufs=1, space="PSUM")
            bcp = bcp_cm.__enter__()
            scp = tc.tile_pool(name="sc", bufs=2, space="PSUM")
            scp_p = scp.__enter__()
            fillp_cm = tc.tile_pool(name="fill", bufs=1, space="PSUM")
            fillp = fillp_cm.__enter__()

            pt_store = {}

            def emit_scores(h, jt):
                row = (h % 2) * 64
                pair = h // 2
                qrow = qt_sb[row:row + 64, pair, :]
                krow = kt_sb[row:row + 64, pair, :]
                ic0 = jt // 4
                for pr in range(2):
                    lo = max(ic0, pr * 2)
                    hi = pr * 2 + 2
                    if lo >= hi:
                        continue
                    sc_ps = scp_p.tile([128, 2, CHUNK], f32, tag="sc")
                    pt_sb = ptp.tile([128, 2, CHUNK], bf16, tag="pt")
                    for ic in range(lo, hi):
                        off = max(0, jt * 128 - ic * CHUNK)
                        _mm(nc.tensor.matmul(
                            sc_ps[:, ic - pr * 2, off:CHUNK],
                            krow[:, jt * 128:(jt + 1) * 128],
                            qrow[:, ic * CHUNK + off:(ic + 1) * CHUNK],
                            start=True,
                            stop=True,
                        ))
                    off0 = max(0, jt * 128 - lo * CHUNK)
                    flat_lo = (lo - pr * 2) * CHUNK + off0
                    scf = sc_ps[:].rearrange("p a b -> p (a b)")
                    ptf = pt_sb[:].rearrange("p a b -> p (a b)")
                    nc.scalar.activation(
                        ptf[:, flat_lo:2 * CHUNK],
                        scf[:, flat_lo:2 * CHUNK],
                        EXP,
                        scale=float(SCALE),
                    )
                    if pr * 2 <= ic0 < hi:  # diagonal block
                        nc.vector.tensor_tensor(
                            out=ptf[:, flat_lo:flat_lo + 128],
                            in0=ptf[:, flat_lo:flat_lo + 128],
                            in1=mask_sb[:],
                            op=MUL,
                        )
                    for ic in range(lo, hi):
                        pt_store[(h, jt, ic)] = (pt_sb, ic - pr * 2)

            def emit_ctx_chunk(h, g, key):
                # accumulate chunk g of head h over j-tiles 0..4g+3, then
                # issue the rowsum reciprocal and queue the normalize
                row = (h % 2) * 64
                pair = h // 2
                ctx_ps = ctxp.tile([128, CHUNK], f32, tag="ctx",
                                   name=f"ctx_{h}_{g}")
                for jt in range(0, 4 * g + 4):
                    off = max(0, jt * 128 - g * CHUNK)
                    width = CHUNK - off
                    pt_sb, sub = pt_store[(h, jt, g)]
                    _mm(nc.tensor.matmul(
                        ctx_ps[0:HD + 1, off:off + width],
                        vj_sb[:, jt, h, :],
                        pt_sb[:, sub, off:off + width],
                        start=(jt == 0),
                        stop=(jt == 4 * g + 3),
                    ))
                rs = rsp.tile([1, CHUNK], f32r, tag="rs")
                with nc.allow_low_precision(
                        reason="rowsum reciprocal rounded to f32r"):
                    nc.vector.reciprocal(rs[:], ctx_ps[HD:HD + 1, :])

                def fin(row=row, pair=pair, g=g, ctx_ps=ctx_ps, rs=rs):
                    bc_ps = bcp.tile([HD, CHUNK], f32, tag="bc")
                    _mm(nc.tensor.matmul(
                        bc_ps[:],
                        ones64_sb[:],
                        rs[:],
                        start=True,
                        stop=True,
                    ))
                    bc_sb = bcsbp.tile([HD, CHUNK], f32, tag="bcsb")
                    nc.vector.tensor_copy(bc_sb[:], bc_ps[:])
                    nc.vector.tensor_tensor(
                        out=ctxT2_sb[row:row + 64, pair,
                                     g * CHUNK:(g + 1) * CHUNK],
                        in0=ctx_ps[0:HD, :],
                        in1=bc_sb[:],
                        op=MUL,
                    )

                pending_norms.append((key, fin))

            gg = 0
            for h in range(HPC):
                for g in range(NIC):
                    for jt in range(4 * g, 4 * g + 4):
                        emit_scores(h, jt)
                    if gg < len(filler_units):
                        filler_units[gg]()
                    flush_norm(gg - 1)
                    if g >= 1:
                        emit_ctx_chunk(h, g - 1, gg)
                    elif h >= 1:
                        emit_ctx_chunk(h - 1, NIC - 1, gg)
                    gg += 1
            # last chunk of the last head
            emit_ctx_chunk(HPC - 1, NIC - 1, gg)

            fillp_cm.__exit__(None, None, None)
            scp.__exit__(None, None, None)

            # ---- phase C: output projection (overlaps final norms) --------
            omp_cm = tc.tile_pool(name="om", bufs=5, space="PSUM")
            omp = omp_cm.__enter__()
            osbp_cm = tc.tile_pool(name="osb", bufs=12)
            osbp = osbp_cm.__enter__()

            def emit_outproj(it):
                pso = [omp.tile([128, CHUNK], f32, tag="om",
                                name=f"om_{it}_{nck}")
                       for nck in range(H // CHUNK)]
                for p in range(2):
                    for nck in range(H // CHUNK):
                        _mm(nc.tensor.matmul(
                            pso[nck][:],
                            ctxT2_sb[:, p, it * 128:(it + 1) * 128],
                            wo_sb[:, p, nck * CHUNK:(nck + 1) * CHUNK],
                            start=(p == 0),
                            stop=(p == 1),
                        ))
                for nck in range(H // CHUNK):
                    osb = osbp.tile([128, CHUNK], bf16, tag="osb")
                    if nck == 0:
                        nc.scalar.activation(osb[:], pso[nck][:], COPY)
                    else:
                        nc.vector.tensor_copy(osb[:], pso[nck][:])
                    nc.sync.dma_start(
                        outp.ap()[it * 128:(it + 1) * 128,
                                  nck * CHUNK:(nck + 1) * CHUNK],
                        osb[:],
                    )

            for it in range(8):
                emit_outproj(it)
            flush_norm(10 ** 9)
            for it in range(8, NIT):
                emit_outproj(it)

            osbp_cm.__exit__(None, None, None)
            omp_cm.__exit__(None, None, None)
            bcsbp_cm.__exit__(None, None, None)
            rsp_cm.__exit__(None, None, None)
            ptp_cm.__exit__(None, None, None)
            bcp_cm.__exit__(None, None, None)
            ctxp_cm.__exit__(None, None, None)

    _split_multiwaits(nc)
    return nc


_nc_cache = None


def _get_program():
    global _nc_cache
    if _nc_cache is None:
        _nc_cache = _build_program()
    return _nc_cache


def kernel(hidden_states, Wq, bq, Wk, bk, Wv, bv, Wo, bo):
    from concourse.bass_utils import run_bass_kernel_spmd

    hidden_states = np.asarray(hidden_states, dtype=np.float32)
    Wq, bq = np.asarray(Wq, np.float32), np.asarray(bq, np.float32)
    Wk, bk = np.asarray(Wk, np.float32), np.asarray(bk, np.float32)
    Wv, bv = np.asarray(Wv, np.float32), np.asarray(bv, np.float32)
    Wo, bo = np.asarray(Wo, np.float32), np.asarray(bo, np.float32)

    ones64 = np.ones((1, 64), np.float32)
    # mask[j, i_local] = 1 where query i >= key j inside a diagonal block
    mask = np.tril(np.ones((128, 128), ml_dtypes.bfloat16)).T.copy()
    onescol = np.ones((128, NIT * HPC), ml_dtypes.bfloat16)

    in_maps = []
    for c in range(NCORES):
        b = c // (NCORES // B)
        hg = c % (NCORES // B)
        hsel = slice(hg * HPC * HD, (hg + 1) * HPC * HD)
        xt_c = np.ascontiguousarray(hidden_states[b].T).astype(
            ml_dtypes.bfloat16)
        in_maps.append({
            "xt": xt_c,
            "wq": np.ascontiguousarray(Wq[:, hsel]).astype(ml_dtypes.bfloat16),
            "wk": np.ascontiguousarray(Wk[:, hsel]).astype(ml_dtypes.bfloat16),
            "wv": np.ascontiguousarray(Wv[:, hsel]).astype(ml_dtypes.bfloat16),
            "wo": np.ascontiguousarray(Wo[hsel, :]).astype(ml_dtypes.bfloat16),
            "bq2": bq[hsel].reshape(2, 128).T.copy(),
            "ones64": ones64,
            "mask": mask,
            "onescol": onescol,
        })

    res = run_bass_kernel_spmd(_get_program(), in_maps, list(range(NCORES)))
    out = np.zeros((B, S, H), np.float32)
    for c in range(NCORES):
        out[c // (NCORES // B)] += res.results[c]["outp"].astype(np.float32)
    # exact bias algebra: bk is softmax-invariant; bv and bo fold into a
    # constant row of the output
    out += (bv @ Wo + bo)[None, None, :]
    return out
